# revision 27
# baseline (speedup 1.0000x reference)
"""CustomLSTMCell fused kernel for 8x Trainium2 NeuronCores.

Reference computation (B=8192, D=H=1024):
    z = e_t @ W_x.T + h_prev @ W_h.T + (b_x + b_h + b_extra)   # [B, 4H]
    f, i, o, c = split(z, 4)
    c_t = sigmoid(f) * c_prev + sigmoid(i) * tanh(c)
    h_t = sigmoid(o) * tanh(c_t)

Sharding: 2-way batch x 4-way hidden-unit (8 cores, no collectives).
Each core computes z transposed ([gate_rows, batch] layout) so the bias
folds into the ScalarE activation's per-partition bias operand, and both
matmul operands arrive pre-transposed from the host (contraction dim on
partitions).

Matmuls run in bf16 (max-rel-err ~6e-3, well under the 2e-2 gate).
vs the earlier fp32r version this halves all input DMA bytes (faster
startup chase, no mid-kernel stream stalls) and enables the compiler's
Fast Weight Load path (fp32 dtypes disable FWL): LDWEIGHTS drops
187->97ns and hides fully, so the stream runs at the 216ns/MM
roofline (512 moving cols / 2.4GHz + NX issue).

Measured timeline on a clean (unthrottled) run, ~240us total:
  ~7.3us Tile prologue (fixed) | warmup+first-chunk wait to ~12.6 |
  221.2us gap-free matmul stream at 216ns/MM | ~2.9us elementwise
  tail + final DMA | ~3us Tile drain (fixed).  The chip sometimes
  sits in a P0 power state (PE at 2.0GHz instead of 2.4 -> stream
  stretches to 259ns/MM, ~287us total); that is environmental.

Schedule notes (from ntff traces; see also project memory):
 - An 11-matmul warmup on a zeroed tile keeps the PE busy from ~8us so
   the HAM clock gate's 3.4us SHORT window is paid during the DMA wait;
   it must merge gap-free into the real stream or the window resets and
   ~10 real matmuls run at 1.2GHz.
 - W and the first batch-block's x are DMAed as interleaved per-k-chunk
   tiles on the sync HWDGE ring; the first batch block runs its matmuls
   k-outer so the PE chases the arrival stream.  bias + cprev0/1 ride
   the sync ring BEHIND the stream: on gpsimd they issue immediately and
   the SDMA engines' packet round-robin between the gpsimd and sync
   rings steals half the early HBM bandwidth (measured ~4us of PE
   starvation); a slow DMA on the scalar ring instead poisons one of
   the 8 shared HWDGE completion-sem lanes and stalls the stream's
   lane rotation.
 - DMA chains on the scalar engine develop issue-blocking sem waits that
   delay ACTIVATEs (which gate PSUM-bank release), so the steady-state
   outputs and cprev2+ go on gpsimd (SWDGE), naturally gated behind the
   previous block's output DMAs.
 - h_t/c_t are written as bf16 (host upcasts) -- halves output bytes and
   the final h_t transfer is on the critical path after the last matmul.
 - The last quadruple runs as two half-width (N=256) matmul groups in
   separate PSUM banks, so half 0's elementwise+outputs overlap half 1's
   ~7us of matmuls; the post-last-matmul chain is just o-gate ACT ->
   h_t mul -> one 64KB DMA on the sync ring (the scalar ring has ~0.7us
   first-byte latency).
"""

import sys

if "/opt/trn_rl_repo" not in sys.path:
    sys.path.insert(0, "/opt/trn_rl_repo")

import ml_dtypes
import numpy as np

import concourse.bass as bass
import concourse.mybir as mybir
from concourse import bacc
from concourse.bass_utils import run_bass_kernel_spmd
from concourse.tile import TileContext

F32 = mybir.dt.float32
BF16 = mybir.dt.bfloat16
AFT = mybir.ActivationFunctionType
ALU = mybir.AluOpType

B, D, H = 8192, 1024, 1024
M_BATCH, M_UNIT = 2, 4          # batch split x unit split = 8 cores
BS = B // M_BATCH               # 4096 batch rows per core
U = H // M_UNIT                 # 256 hidden units per core
K = D + H                       # 2048 contraction (e_t | h_prev)
KT = K // 128                   # 16 k-chunks
G = 4 * U                       # 1024 gate rows per core (f|i|o|c x U)
BBLK = 512                      # moving free-dim per matmul
NBB = BS // BBLK                # 8 batch blocks
NJ = U // 128                   # 2 unit sub-blocks of 128 partitions

GATE_FUNCS = [AFT.Sigmoid, AFT.Sigmoid, AFT.Sigmoid, AFT.Tanh]  # f, i, o, c

N_WARMUP = 9


def _build_nc():
    nc = bacc.Bacc()

    xT = nc.dram_tensor("xT", [K, BS], BF16, kind="ExternalInput")
    wT = nc.dram_tensor("wT", [K, G], BF16, kind="ExternalInput")
    bias = nc.dram_tensor("bias", [128, 4 * NJ], F32, kind="ExternalInput")
    cT = nc.dram_tensor("cT", [U, BS], F32, kind="ExternalInput")
    # Outputs in bf16: adds ~2^-9 rounding (total err stays ~6e-3 vs the
    # 2e-2 gate) and halves output DMA bytes -- the final h_t transfer
    # is on the critical path after the last matmul.
    hT_out = nc.dram_tensor("hT_out", [U, BS], BF16, kind="ExternalOutput")
    cT_out = nc.dram_tensor("cT_out", [U, BS], BF16, kind="ExternalOutput")

    xT_r = xT.ap().rearrange("(k p) b -> p k b", p=128)      # [128, KT, BS]
    wT_r = wT.ap().rearrange("(k p) g -> p k g", p=128)      # [128, KT, G]
    cT_r = cT.ap().rearrange("(j p) b -> p j b", p=128)      # [128, NJ, BS]
    hT_r = hT_out.ap().rearrange("(j p) b -> p j b", p=128)
    cTo_r = cT_out.ap().rearrange("(j p) b -> p j b", p=128)

    with TileContext(nc) as tc:
        with (
            tc.tile_pool(name="wpool", bufs=1) as wpool,
            tc.tile_pool(name="xpool", bufs=2) as xpool,
            tc.tile_pool(name="cpool", bufs=2) as cpool,
            tc.tile_pool(name="gpool", bufs=2) as gpool,
            tc.tile_pool(name="opool", bufs=2) as opool,
            tc.tile_pool(name="psum", bufs=2, space="PSUM") as pp,
        ):
            # Bias arrives pre-tiled [128, 4*NJ] from the host (one 32B
            # row per partition, vs a 4096x4B gather).
            bias_sb = wpool.tile([128, 4 * NJ], F32)

            # PE warm-up on a zeroed scratch tile while the first W/x
            # chunks are in flight: trips the HAM clock gate's SHORT
            # window (~3.4us of sustained PE activity) early, so real
            # matmuls run at 2.4GHz from the first chunk.  The memset
            # runs on gpsimd, which exits the Tile prologue earliest.
            warm = wpool.tile([128, BBLK], BF16, name="warm")
            nc.gpsimd.memset(warm[:], 0.0)
            warm_ps = pp.tile([128, BBLK], F32, tag="ps0", name="warm_ps")
            for _ in range(N_WARMUP):
                nc.tensor.matmul(
                    warm_ps[:], warm[:, 0:128], warm[:], start=True, stop=True
                )

            # Per-k W tiles, DMA-interleaved with the first batch block's
            # x tiles so the PE can start on chunk 0 immediately and
            # chase the arrival stream.  Everything on the sync HWDGE
            # ring: its ~650ns/DMA issue rate paces arrivals in k-order;
            # the scalar ring is both slow (~55GB/s measured) and shares
            # the HWDGE sem-lane pool, and the scalar engine must stay
            # free for ACTIVATEs.  w0 lands as two halves, gate-hi half
            # first, so chunk 0's (c,o)-gate matmuls can start before
            # the (f,i) half arrives.
            w_sb = []
            x0_sb = []
            for k in range(KT):
                wt = wpool.tile([128, G], BF16, tag=f"w{k}", name=f"w{k}")
                if k == 0:
                    # x00 + w0-hi prefetch on the gpsimd ring (exits the
                    # prologue earliest, right after the warmup memset):
                    # a second independent path to first-chunk data, in
                    # parallel with the sync ring's w0-lo.  Only 256KB,
                    # so the packet round-robin theft from the sync
                    # stream ends before the chase needs full rate.
                    xp = xpool.tile(
                        [128, 2, BBLK], BF16, tag="xp0", name="xp0", bufs=3
                    )
                    nc.gpsimd.dma_start(out=xp[:, 0, :], in_=xT_r[:, 0, 0:BBLK])
                    nc.gpsimd.dma_start(out=wt[:, G // 2:], in_=wT_r[:, 0, G // 2:])
                    nc.sync.dma_start(out=wt[:, 0:G // 2], in_=wT_r[:, 0, 0:G // 2])
                else:
                    nc.sync.dma_start(out=wt[:], in_=wT_r[:, k, :])
                    if k % 2 == 0:
                        xp = xpool.tile(
                            [128, 2, BBLK], BF16, tag=f"xp{k // 2}", name=f"xp{k // 2}", bufs=3
                        )
                    nc.sync.dma_start(out=xp[:, k % 2, :], in_=xT_r[:, k, 0:BBLK])
                w_sb.append(wt)
                x0_sb.append(xp[:, k % 2, :])

            # bias + the first two cprev blocks ride the sync ring
            # BEHIND the W/x stream (engine FIFO defers them past the
            # chase window).  On gpsimd they would issue immediately and
            # the SDMA engines' packet round-robin between the gpsimd
            # and sync rings would steal ~half the early HBM bandwidth
            # exactly while the PE chases the k-stream (measured ~4us of
            # starvation).  cprev2+ stay on gpsimd, naturally gated
            # behind the previous block's output DMAs.
            nc.sync.dma_start(out=bias_sb[:], in_=bias.ap())

            def load_cprev(bb, ring=None):
                t = cpool.tile([128, NJ, BBLK], F32, tag="cprev", name="cprev")
                (ring or nc.gpsimd).dma_start(
                    out=t[:], in_=cT_r[:, :, bb * BBLK:(bb + 1) * BBLK]
                )
                return t

            def elementwise(ps, cprev_sb, bb, j, n_split=1, out_rings=None,
                            width=BBLK, coff=0):
                """Gate nonlinearities + cell update for one quadruple.

                ps[g] columns [0, width) correspond to batch-block
                columns [coff, coff+width).  out_rings: (ct, ht) ring
                per split chunk; default both gpsimd (SWDGE), fully
                overlapped in steady state.
                """
                w = width // n_split
                for s in range(n_split):
                    ct_ring, ht_ring = (
                        out_rings[s] if out_rings else (nc.gpsimd, nc.gpsimd)
                    )
                    psl = slice(s * w, (s + 1) * w)
                    csl = slice(coff + s * w, coff + (s + 1) * w)

                    def gate(g):
                        at = gpool.tile([128, w], F32, tag=f"act{g}", name=f"act{g}")
                        nc.scalar.activation(
                            at[:], ps[g][:, psl], GATE_FUNCS[g],
                            bias=bias_sb[:, 2 * g + j: 2 * g + j + 1],
                        )
                        return at

                    # ACT stream order mirrors the (c,f,i,o) matmul order so
                    # the c_t chain completes before the o-gate's matmuls do.
                    mc = gate(3)
                    gf = gate(0)
                    gi = gate(1)
                    t1 = gpool.tile([128, w], F32, tag="t1", name="t1")
                    nc.vector.tensor_tensor(
                        t1[:], gf[:], cprev_sb[:, j, csl], ALU.mult
                    )
                    t2 = gpool.tile([128, w], F32, tag="t2", name="t2")
                    nc.vector.tensor_tensor(t2[:], gi[:], mc[:], ALU.mult)
                    ct = opool.tile([128, w], BF16, tag="ct", name="ct")
                    nc.vector.tensor_tensor(ct[:], t1[:], t2[:], ALU.add)
                    th = gpool.tile([128, w], F32, tag="th", name="th")
                    nc.scalar.activation(th[:], ct[:], AFT.Tanh)
                    go = gate(2)
                    ht = opool.tile([128, w], BF16, tag="ht", name="ht")
                    nc.vector.tensor_tensor(ht[:], go[:], th[:], ALU.mult)

                    osl = slice(bb * BBLK + coff + s * w,
                                bb * BBLK + coff + (s + 1) * w)
                    ct_ring.dma_start(out=cTo_r[:, j, osl], in_=ct[:])
                    ht_ring.dma_start(out=hT_r[:, j, osl], in_=ht[:])

            # ---- batch block 0: k-outer over both j's, chasing the DMA
            # stream (8 MMs per arriving k-chunk vs ~1.4us/chunk
            # delivery) ----
            cprev0 = load_cprev(0, ring=nc.sync)
            ps0 = [
                [pp.tile([128, BBLK], F32, tag=f"ps{g}", name=f"ps{g}") for g in range(4)]
                for j in range(NJ)
            ]
            for k in range(KT):
                # chunk 0 runs j-inner with the upper-half gates (c,o)
                # first, matching the two-half w0 arrival order: the
                # first 4 matmuls need only the gpsimd-prefetched w0-hi,
                # buying ~1us for the sync ring's w0-lo to land.
                # Elsewhere c-gate first (longest elementwise chain).
                order = (
                    [(g, j) for g in (3, 2, 0, 1) for j in range(NJ)]
                    if k == 0 else
                    [(g, j) for j in range(NJ) for g in (3, 0, 1, 2)]
                )
                for g, j in order:
                    nc.tensor.matmul(
                        ps0[j][g][:],
                        w_sb[k][:, g * U + j * 128: g * U + (j + 1) * 128],
                        x0_sb[k][:],
                        start=(k == 0),
                        stop=(k == KT - 1),
                    )
            for j in range(NJ):
                elementwise(ps0[j], cprev0, 0, j)

            # ---- batch blocks 1..NBB-1: gate-outer, k-inner ----
            for bb in range(1, NBB):
                bsl = slice(bb * BBLK, (bb + 1) * BBLK)
                x_sb = []
                for kp in range(KT // 2):
                    xt = xpool.tile(
                        [128, 2, BBLK], BF16, tag=f"xp{kp}", name=f"xp{kp}", bufs=3
                    )
                    nc.sync.dma_start(out=xt[:], in_=xT_r[:, 2 * kp:2 * kp + 2, bsl])
                    x_sb.extend([xt[:, 0, :], xt[:, 1, :]])
                cprev_sb = load_cprev(bb, ring=nc.sync if bb == 1 else None)

                for j in range(NJ):
                    last = (bb == NBB - 1) and (j == NJ - 1)
                    if not last:
                        ps = [None] * 4
                        for g in (3, 0, 1, 2):  # c-gate first: longest chain
                            pst = pp.tile([128, BBLK], F32, tag=f"ps{g}", name=f"ps{g}")
                            col0 = g * U + j * 128
                            for k in range(KT):
                                nc.tensor.matmul(
                                    pst[:],
                                    w_sb[k][:, col0:col0 + 128],
                                    x_sb[k][:],
                                    start=(k == 0),
                                    stop=(k == KT - 1),
                                )
                            ps[g] = pst
                        elementwise(ps, cprev_sb, bb, j)
                        continue

                    # Final quadruple: two half-width (N=256) matmul
                    # groups in separate PSUM banks, so half 0's whole
                    # elementwise+output chain overlaps half 1's ~7us of
                    # matmuls and the post-last-matmul tail is just the
                    # o-gate ACT -> h_t mul -> one 64KB DMA.  ht rings
                    # split across sync/scalar so the two ~650ns DMA
                    # issues don't serialize.
                    hw = BBLK // 2
                    for h in range(2):
                        ps = [None] * 4
                        for g in (3, 0, 1, 2):
                            pst = pp.tile(
                                [128, BBLK], F32, tag=f"ps{g}", name=f"ps{g}h{h}"
                            )
                            col0 = g * U + j * 128
                            for k in range(KT):
                                nc.tensor.matmul(
                                    pst[:, 0:hw],
                                    w_sb[k][:, col0:col0 + 128],
                                    x_sb[k][:, h * hw:(h + 1) * hw],
                                    start=(k == 0),
                                    stop=(k == KT - 1),
                                )
                            ps[g] = pst
                        elementwise(
                            ps, cprev_sb, bb, j,
                            out_rings=[(nc.gpsimd, nc.scalar if h == 0 else nc.sync)],
                            width=hw, coff=h * hw,
                        )

    nc.finalize()
    return nc


def _shard_inputs(e_t, h_prev, c_prev, W_x, b_x, W_h, b_h, b_extra):
    e_t = np.asarray(e_t, dtype=np.float32)
    h_prev = np.asarray(h_prev, dtype=np.float32)
    c_prev = np.ascontiguousarray(np.asarray(c_prev, dtype=np.float32))
    W_x = np.asarray(W_x, dtype=np.float32)
    W_h = np.asarray(W_h, dtype=np.float32)
    bias_full = (
        np.asarray(b_x, dtype=np.float32)
        + np.asarray(b_h, dtype=np.float32)
        + np.asarray(b_extra, dtype=np.float32)
    )

    # X^T = [e_t | h_prev]^T : [K, B] in bf16
    XT = np.empty((K, B), dtype=ml_dtypes.bfloat16)
    XT[:D] = e_t.astype(ml_dtypes.bfloat16).T
    XT[D:] = h_prev.astype(ml_dtypes.bfloat16).T
    W = np.concatenate([W_x, W_h], axis=1).astype(ml_dtypes.bfloat16)  # [4H, K]

    in_maps = []
    for core in range(M_BATCH * M_UNIT):
        m, q = divmod(core, M_UNIT)
        rows = np.concatenate(
            [np.arange(g0 + q * U, g0 + (q + 1) * U) for g0 in (0, H, 2 * H, 3 * H)]
        )
        bias_tiled = np.ascontiguousarray(
            bias_full[rows].reshape(4, NJ, 128).transpose(2, 0, 1).reshape(128, 4 * NJ)
        )
        in_maps.append({
            "xT": np.ascontiguousarray(XT[:, m * BS:(m + 1) * BS]),
            "wT": np.ascontiguousarray(W[rows].T),
            "bias": bias_tiled,
            "cT": np.ascontiguousarray(c_prev[m * BS:(m + 1) * BS, q * U:(q + 1) * U].T),
        })
    return in_maps


def _assemble_outputs(results):
    h_t = np.empty((B, H), dtype=np.float32)
    c_t = np.empty((B, H), dtype=np.float32)
    for core, res in enumerate(results):
        m, q = divmod(core, M_UNIT)
        h_t[m * BS:(m + 1) * BS, q * U:(q + 1) * U] = np.asarray(
            res["hT_out"], dtype=np.float32).T
        c_t[m * BS:(m + 1) * BS, q * U:(q + 1) * U] = np.asarray(
            res["cT_out"], dtype=np.float32).T
    return h_t, c_t


def kernel(e_t, h_prev, c_prev, W_x, b_x, W_h, b_h, b_extra, _runner=None):
    in_maps = _shard_inputs(e_t, h_prev, c_prev, W_x, b_x, W_h, b_h, b_extra)
    nc = _build_nc()
    if _runner is None:
        res = run_bass_kernel_spmd(nc, in_maps, core_ids=list(range(8)))
        results = res.results
    else:
        results = _runner(nc, in_maps)
    return _assemble_outputs(results)


# revision 28
# speedup vs baseline: 1.0088x; 1.0088x over previous
"""CustomLSTMCell fused kernel for 8x Trainium2 NeuronCores.

Reference computation (B=8192, D=H=1024):
    z = e_t @ W_x.T + h_prev @ W_h.T + (b_x + b_h + b_extra)   # [B, 4H]
    f, i, o, c = split(z, 4)
    c_t = sigmoid(f) * c_prev + sigmoid(i) * tanh(c)
    h_t = sigmoid(o) * tanh(c_t)

Sharding: 2-way batch x 4-way hidden-unit (8 cores, no collectives).
Each core computes z transposed ([gate_rows, batch] layout) so the bias
folds into the ScalarE activation's per-partition bias operand, and both
matmul operands arrive pre-transposed from the host (contraction dim on
partitions).

Matmuls run in bf16 (max-rel-err ~6e-3, well under the 2e-2 gate).
vs the earlier fp32r version this halves all input DMA bytes (faster
startup chase, no mid-kernel stream stalls) and enables the compiler's
Fast Weight Load path (fp32 dtypes disable FWL): LDWEIGHTS drops
187->97ns and hides fully, so the stream runs at the 216ns/MM
roofline (512 moving cols / 2.4GHz + NX issue).

Measured timeline on a clean (unthrottled) run, ~240us total:
  ~7.3us Tile prologue (fixed) | warmup+first-chunk wait to ~12.6 |
  221.2us gap-free matmul stream at 216ns/MM | ~2.9us elementwise
  tail + final DMA | ~3us Tile drain (fixed).  The chip sometimes
  sits in a P0 power state (PE at 2.0GHz instead of 2.4 -> stream
  stretches to 259ns/MM, ~287us total); that is environmental.

Schedule notes (from ntff traces; see also project memory):
 - An 11-matmul warmup on a zeroed tile keeps the PE busy from ~8us so
   the HAM clock gate's 3.4us SHORT window is paid during the DMA wait;
   it must merge gap-free into the real stream or the window resets and
   ~10 real matmuls run at 1.2GHz.
 - W and the first batch-block's x are DMAed as interleaved per-k-chunk
   tiles on the sync HWDGE ring; the first batch block runs its matmuls
   k-outer so the PE chases the arrival stream.  bias + cprev0/1 ride
   the sync ring BEHIND the stream: on gpsimd they issue immediately and
   the SDMA engines' packet round-robin between the gpsimd and sync
   rings steals half the early HBM bandwidth (measured ~4us of PE
   starvation); a slow DMA on the scalar ring instead poisons one of
   the 8 shared HWDGE completion-sem lanes and stalls the stream's
   lane rotation.
 - DMA chains on the scalar engine develop issue-blocking sem waits that
   delay ACTIVATEs (which gate PSUM-bank release), so the steady-state
   outputs and cprev2+ go on gpsimd (SWDGE), naturally gated behind the
   previous block's output DMAs.
 - h_t/c_t are written as bf16 (host upcasts) -- halves output bytes and
   the final h_t transfer is on the critical path after the last matmul.
 - The last quadruple runs as two half-width (N=256) matmul groups in
   separate PSUM banks, so half 0's elementwise+outputs overlap half 1's
   ~7us of matmuls; the post-last-matmul chain is just o-gate ACT ->
   h_t mul -> one 64KB DMA on the sync ring (the scalar ring has ~0.7us
   first-byte latency).
"""

import sys

if "/opt/trn_rl_repo" not in sys.path:
    sys.path.insert(0, "/opt/trn_rl_repo")

import ml_dtypes
import numpy as np

import concourse.bass as bass
import concourse.mybir as mybir
from concourse import bacc
from concourse.bass_utils import run_bass_kernel_spmd
from concourse.tile import TileContext

F32 = mybir.dt.float32
BF16 = mybir.dt.bfloat16
AFT = mybir.ActivationFunctionType
ALU = mybir.AluOpType

B, D, H = 8192, 1024, 1024
M_BATCH, M_UNIT = 2, 4          # batch split x unit split = 8 cores
BS = B // M_BATCH               # 4096 batch rows per core
U = H // M_UNIT                 # 256 hidden units per core
K = D + H                       # 2048 contraction (e_t | h_prev)
KT = K // 128                   # 16 k-chunks
G = 4 * U                       # 1024 gate rows per core (f|i|o|c x U)
BBLK = 512                      # moving free-dim per matmul
NBB = BS // BBLK                # 8 batch blocks
NJ = U // 128                   # 2 unit sub-blocks of 128 partitions

GATE_FUNCS = [AFT.Sigmoid, AFT.Sigmoid, AFT.Sigmoid, AFT.Tanh]  # f, i, o, c

N_WARMUP = 11


def _build_nc():
    nc = bacc.Bacc()

    xT = nc.dram_tensor("xT", [K, BS], BF16, kind="ExternalInput")
    wT = nc.dram_tensor("wT", [K, G], BF16, kind="ExternalInput")
    bias = nc.dram_tensor("bias", [128, 4 * NJ], F32, kind="ExternalInput")
    cT = nc.dram_tensor("cT", [U, BS], F32, kind="ExternalInput")
    # Outputs in bf16: adds ~2^-9 rounding (total err stays ~6e-3 vs the
    # 2e-2 gate) and halves output DMA bytes -- the final h_t transfer
    # is on the critical path after the last matmul.
    hT_out = nc.dram_tensor("hT_out", [U, BS], BF16, kind="ExternalOutput")
    cT_out = nc.dram_tensor("cT_out", [U, BS], BF16, kind="ExternalOutput")

    xT_r = xT.ap().rearrange("(k p) b -> p k b", p=128)      # [128, KT, BS]
    wT_r = wT.ap().rearrange("(k p) g -> p k g", p=128)      # [128, KT, G]
    cT_r = cT.ap().rearrange("(j p) b -> p j b", p=128)      # [128, NJ, BS]
    hT_r = hT_out.ap().rearrange("(j p) b -> p j b", p=128)
    cTo_r = cT_out.ap().rearrange("(j p) b -> p j b", p=128)

    with TileContext(nc) as tc:
        with (
            tc.tile_pool(name="wpool", bufs=1) as wpool,
            tc.tile_pool(name="xpool", bufs=2) as xpool,
            tc.tile_pool(name="cpool", bufs=2) as cpool,
            tc.tile_pool(name="gpool", bufs=2) as gpool,
            tc.tile_pool(name="opool", bufs=2) as opool,
            tc.tile_pool(name="psum", bufs=2, space="PSUM") as pp,
        ):
            # Bias arrives pre-tiled [128, 4*NJ] from the host (one 32B
            # row per partition, vs a 4096x4B gather).
            bias_sb = wpool.tile([128, 4 * NJ], F32)

            # PE warm-up on a zeroed scratch tile while the first W/x
            # chunks are in flight: trips the HAM clock gate's SHORT
            # window (~3.4us of sustained PE activity) early, so real
            # matmuls run at 2.4GHz from the first chunk.  The memset
            # runs on gpsimd, which exits the Tile prologue earliest.
            warm = wpool.tile([128, BBLK], BF16, name="warm")
            nc.gpsimd.memset(warm[:], 0.0)
            warm_ps = pp.tile([128, BBLK], F32, tag="ps0", name="warm_ps")
            for _ in range(N_WARMUP):
                nc.tensor.matmul(
                    warm_ps[:], warm[:, 0:128], warm[:], start=True, stop=True
                )

            # Per-k W tiles, DMA-interleaved with the first batch block's
            # x tiles so the PE can start on chunk 0 immediately and
            # chase the arrival stream.  Everything on the sync HWDGE
            # ring: its ~650ns/DMA issue rate paces arrivals in k-order;
            # the scalar ring is both slow (~55GB/s measured) and shares
            # the HWDGE sem-lane pool, and the scalar engine must stay
            # free for ACTIVATEs.  w0 lands as two halves, gate-hi half
            # first, so chunk 0's (c,o)-gate matmuls can start before
            # the (f,i) half arrives.
            w_sb = []
            x0_sb = []
            for k in range(KT):
                wt = wpool.tile([128, G], BF16, tag=f"w{k}", name=f"w{k}")
                if k == 0:
                    xp = xpool.tile(
                        [128, 2, BBLK], BF16, tag="xp0", name="xp0", bufs=3
                    )
                    nc.sync.dma_start(out=xp[:, 0, :], in_=xT_r[:, 0, 0:BBLK])
                    nc.sync.dma_start(out=wt[:, G // 2:], in_=wT_r[:, 0, G // 2:])
                    nc.sync.dma_start(out=wt[:, 0:G // 2], in_=wT_r[:, 0, 0:G // 2])
                else:
                    nc.sync.dma_start(out=wt[:], in_=wT_r[:, k, :])
                    if k % 2 == 0:
                        xp = xpool.tile(
                            [128, 2, BBLK], BF16, tag=f"xp{k // 2}", name=f"xp{k // 2}", bufs=3
                        )
                    nc.sync.dma_start(out=xp[:, k % 2, :], in_=xT_r[:, k, 0:BBLK])
                w_sb.append(wt)
                x0_sb.append(xp[:, k % 2, :])

            # bias + the first two cprev blocks ride the sync ring
            # BEHIND the W/x stream (engine FIFO defers them past the
            # chase window).  On gpsimd they would issue immediately and
            # the SDMA engines' packet round-robin between the gpsimd
            # and sync rings would steal ~half the early HBM bandwidth
            # exactly while the PE chases the k-stream (measured ~4us of
            # starvation).  cprev2+ stay on gpsimd, naturally gated
            # behind the previous block's output DMAs.
            nc.sync.dma_start(out=bias_sb[:], in_=bias.ap())

            def load_cprev(bb, ring=None):
                t = cpool.tile([128, NJ, BBLK], F32, tag="cprev", name="cprev")
                (ring or nc.gpsimd).dma_start(
                    out=t[:], in_=cT_r[:, :, bb * BBLK:(bb + 1) * BBLK]
                )
                return t

            def elementwise(ps, cprev_sb, bb, j, n_split=1, out_rings=None,
                            width=BBLK, coff=0):
                """Gate nonlinearities + cell update for one quadruple.

                ps[g] columns [0, width) correspond to batch-block
                columns [coff, coff+width).  out_rings: (ct, ht) ring
                per split chunk; default both gpsimd (SWDGE), fully
                overlapped in steady state.
                """
                w = width // n_split
                for s in range(n_split):
                    ct_ring, ht_ring = (
                        out_rings[s] if out_rings else (nc.gpsimd, nc.gpsimd)
                    )
                    psl = slice(s * w, (s + 1) * w)
                    csl = slice(coff + s * w, coff + (s + 1) * w)

                    def gate(g):
                        at = gpool.tile([128, w], F32, tag=f"act{g}", name=f"act{g}")
                        nc.scalar.activation(
                            at[:], ps[g][:, psl], GATE_FUNCS[g],
                            bias=bias_sb[:, 2 * g + j: 2 * g + j + 1],
                        )
                        return at

                    # ACT stream order mirrors the (c,f,i,o) matmul order so
                    # the c_t chain completes before the o-gate's matmuls do.
                    mc = gate(3)
                    gf = gate(0)
                    gi = gate(1)
                    t1 = gpool.tile([128, w], F32, tag="t1", name="t1")
                    nc.vector.tensor_tensor(
                        t1[:], gf[:], cprev_sb[:, j, csl], ALU.mult
                    )
                    t2 = gpool.tile([128, w], F32, tag="t2", name="t2")
                    nc.vector.tensor_tensor(t2[:], gi[:], mc[:], ALU.mult)
                    ct = opool.tile([128, w], BF16, tag="ct", name="ct")
                    nc.vector.tensor_tensor(ct[:], t1[:], t2[:], ALU.add)
                    th = gpool.tile([128, w], F32, tag="th", name="th")
                    nc.scalar.activation(th[:], ct[:], AFT.Tanh)
                    go = gate(2)
                    ht = opool.tile([128, w], BF16, tag="ht", name="ht")
                    nc.vector.tensor_tensor(ht[:], go[:], th[:], ALU.mult)

                    osl = slice(bb * BBLK + coff + s * w,
                                bb * BBLK + coff + (s + 1) * w)
                    ct_ring.dma_start(out=cTo_r[:, j, osl], in_=ct[:])
                    ht_ring.dma_start(out=hT_r[:, j, osl], in_=ht[:])

            # ---- batch block 0: k-outer over both j's, chasing the DMA
            # stream (8 MMs per arriving k-chunk vs ~1.4us/chunk
            # delivery) ----
            cprev0 = load_cprev(0, ring=nc.sync)
            ps0 = [
                [pp.tile([128, BBLK], F32, tag=f"ps{g}", name=f"ps{g}") for g in range(4)]
                for j in range(NJ)
            ]
            for k in range(KT):
                # chunk 0 runs the upper-half gates (c,o) first to match
                # the two-half w0 arrival order; elsewhere c-gate first
                # (longest elementwise chain).
                gate_order = (3, 2, 0, 1) if k == 0 else (3, 0, 1, 2)
                for j in range(NJ):
                    for g in gate_order:
                        nc.tensor.matmul(
                            ps0[j][g][:],
                            w_sb[k][:, g * U + j * 128: g * U + (j + 1) * 128],
                            x0_sb[k][:],
                            start=(k == 0),
                            stop=(k == KT - 1),
                        )
            for j in range(NJ):
                elementwise(ps0[j], cprev0, 0, j)

            # ---- batch blocks 1..NBB-1: gate-outer, k-inner ----
            for bb in range(1, NBB):
                bsl = slice(bb * BBLK, (bb + 1) * BBLK)
                x_sb = []
                for kp in range(KT // 2):
                    xt = xpool.tile(
                        [128, 2, BBLK], BF16, tag=f"xp{kp}", name=f"xp{kp}", bufs=3
                    )
                    nc.sync.dma_start(out=xt[:], in_=xT_r[:, 2 * kp:2 * kp + 2, bsl])
                    x_sb.extend([xt[:, 0, :], xt[:, 1, :]])
                cprev_sb = load_cprev(bb, ring=nc.sync if bb == 1 else None)

                for j in range(NJ):
                    last = (bb == NBB - 1) and (j == NJ - 1)
                    if not last:
                        ps = [None] * 4
                        for g in (3, 0, 1, 2):  # c-gate first: longest chain
                            pst = pp.tile([128, BBLK], F32, tag=f"ps{g}", name=f"ps{g}")
                            col0 = g * U + j * 128
                            for k in range(KT):
                                nc.tensor.matmul(
                                    pst[:],
                                    w_sb[k][:, col0:col0 + 128],
                                    x_sb[k][:],
                                    start=(k == 0),
                                    stop=(k == KT - 1),
                                )
                            ps[g] = pst
                        elementwise(ps, cprev_sb, bb, j)
                        continue

                    # Final quadruple: two half-width (N=256) matmul
                    # groups in separate PSUM banks, so half 0's whole
                    # elementwise+output chain overlaps half 1's ~7us of
                    # matmuls and the post-last-matmul tail is just the
                    # o-gate ACT -> h_t mul -> one 64KB DMA.  ht rings
                    # split across sync/scalar so the two ~650ns DMA
                    # issues don't serialize.
                    hw = BBLK // 2
                    for h in range(2):
                        ps = [None] * 4
                        for g in (3, 0, 1, 2):
                            pst = pp.tile(
                                [128, BBLK], F32, tag=f"ps{g}", name=f"ps{g}h{h}"
                            )
                            col0 = g * U + j * 128
                            for k in range(KT):
                                nc.tensor.matmul(
                                    pst[:, 0:hw],
                                    w_sb[k][:, col0:col0 + 128],
                                    x_sb[k][:, h * hw:(h + 1) * hw],
                                    start=(k == 0),
                                    stop=(k == KT - 1),
                                )
                            ps[g] = pst
                        elementwise(
                            ps, cprev_sb, bb, j,
                            out_rings=[(nc.gpsimd, nc.scalar if h == 0 else nc.sync)],
                            width=hw, coff=h * hw,
                        )

    nc.finalize()
    return nc


def _shard_inputs(e_t, h_prev, c_prev, W_x, b_x, W_h, b_h, b_extra):
    e_t = np.asarray(e_t, dtype=np.float32)
    h_prev = np.asarray(h_prev, dtype=np.float32)
    c_prev = np.ascontiguousarray(np.asarray(c_prev, dtype=np.float32))
    W_x = np.asarray(W_x, dtype=np.float32)
    W_h = np.asarray(W_h, dtype=np.float32)
    bias_full = (
        np.asarray(b_x, dtype=np.float32)
        + np.asarray(b_h, dtype=np.float32)
        + np.asarray(b_extra, dtype=np.float32)
    )

    # X^T = [e_t | h_prev]^T : [K, B] in bf16
    XT = np.empty((K, B), dtype=ml_dtypes.bfloat16)
    XT[:D] = e_t.astype(ml_dtypes.bfloat16).T
    XT[D:] = h_prev.astype(ml_dtypes.bfloat16).T
    W = np.concatenate([W_x, W_h], axis=1).astype(ml_dtypes.bfloat16)  # [4H, K]

    in_maps = []
    for core in range(M_BATCH * M_UNIT):
        m, q = divmod(core, M_UNIT)
        rows = np.concatenate(
            [np.arange(g0 + q * U, g0 + (q + 1) * U) for g0 in (0, H, 2 * H, 3 * H)]
        )
        bias_tiled = np.ascontiguousarray(
            bias_full[rows].reshape(4, NJ, 128).transpose(2, 0, 1).reshape(128, 4 * NJ)
        )
        in_maps.append({
            "xT": np.ascontiguousarray(XT[:, m * BS:(m + 1) * BS]),
            "wT": np.ascontiguousarray(W[rows].T),
            "bias": bias_tiled,
            "cT": np.ascontiguousarray(c_prev[m * BS:(m + 1) * BS, q * U:(q + 1) * U].T),
        })
    return in_maps


def _assemble_outputs(results):
    h_t = np.empty((B, H), dtype=np.float32)
    c_t = np.empty((B, H), dtype=np.float32)
    for core, res in enumerate(results):
        m, q = divmod(core, M_UNIT)
        h_t[m * BS:(m + 1) * BS, q * U:(q + 1) * U] = np.asarray(
            res["hT_out"], dtype=np.float32).T
        c_t[m * BS:(m + 1) * BS, q * U:(q + 1) * U] = np.asarray(
            res["cT_out"], dtype=np.float32).T
    return h_t, c_t


def kernel(e_t, h_prev, c_prev, W_x, b_x, W_h, b_h, b_extra, _runner=None):
    in_maps = _shard_inputs(e_t, h_prev, c_prev, W_x, b_x, W_h, b_h, b_extra)
    nc = _build_nc()
    if _runner is None:
        res = run_bass_kernel_spmd(nc, in_maps, core_ids=list(range(8)))
        results = res.results
    else:
        results = _runner(nc, in_maps)
    return _assemble_outputs(results)


# revision 30
# speedup vs baseline: 1.0123x; 1.0035x over previous
"""CustomLSTMCell fused kernel for 8x Trainium2 NeuronCores.

Reference computation (B=8192, D=H=1024):
    z = e_t @ W_x.T + h_prev @ W_h.T + (b_x + b_h + b_extra)   # [B, 4H]
    f, i, o, c = split(z, 4)
    c_t = sigmoid(f) * c_prev + sigmoid(i) * tanh(c)
    h_t = sigmoid(o) * tanh(c_t)

Sharding: 2-way batch x 4-way hidden-unit (8 cores, no collectives).
Each core computes z transposed ([gate_rows, batch] layout) so the bias
folds into the ScalarE activation's per-partition bias operand, and both
matmul operands arrive pre-transposed from the host (contraction dim on
partitions).

Matmuls run in bf16 (max-rel-err ~6e-3, well under the 2e-2 gate).
vs the earlier fp32r version this halves all input DMA bytes (faster
startup chase, no mid-kernel stream stalls) and enables the compiler's
Fast Weight Load path (fp32 dtypes disable FWL): LDWEIGHTS drops
187->97ns and hides fully, so the stream runs at the 216ns/MM
roofline (512 moving cols / 2.4GHz + NX issue).

Measured timeline on a clean (unthrottled) run, ~240us total:
  ~7.3us Tile prologue (fixed) | warmup+first-chunk wait to ~12.6 |
  221.2us gap-free matmul stream at 216ns/MM | ~2.9us elementwise
  tail + final DMA | ~3us Tile drain (fixed).  The chip sometimes
  sits in a P0 power state (PE at 2.0GHz instead of 2.4 -> stream
  stretches to 259ns/MM, ~287us total); that is environmental.

Schedule notes (from ntff traces; see also project memory):
 - An 11-matmul warmup on a zeroed tile keeps the PE busy from ~8us so
   the HAM clock gate's 3.4us SHORT window is paid during the DMA wait;
   it must merge gap-free into the real stream or the window resets and
   ~10 real matmuls run at 1.2GHz.
 - W and the first batch-block's x are DMAed as interleaved per-k-chunk
   tiles on the sync HWDGE ring; the first batch block runs its matmuls
   k-outer so the PE chases the arrival stream.  bias + cprev0/1 ride
   the sync ring BEHIND the stream: on gpsimd they issue immediately and
   the SDMA engines' packet round-robin between the gpsimd and sync
   rings steals half the early HBM bandwidth (measured ~4us of PE
   starvation); a slow DMA on the scalar ring instead poisons one of
   the 8 shared HWDGE completion-sem lanes and stalls the stream's
   lane rotation.
 - DMA chains on the scalar engine develop issue-blocking sem waits that
   delay ACTIVATEs (which gate PSUM-bank release), so the steady-state
   outputs and cprev2+ go on gpsimd (SWDGE), naturally gated behind the
   previous block's output DMAs.
 - h_t/c_t are written as bf16 (host upcasts) -- halves output bytes and
   the final h_t transfer is on the critical path after the last matmul.
 - The last quadruple runs as two half-width (N=256) matmul groups in
   separate PSUM banks, so half 0's elementwise+outputs overlap half 1's
   ~7us of matmuls; the post-last-matmul chain is just o-gate ACT ->
   h_t mul -> one 64KB DMA on the sync ring (the scalar ring has ~0.7us
   first-byte latency).
"""

import sys

if "/opt/trn_rl_repo" not in sys.path:
    sys.path.insert(0, "/opt/trn_rl_repo")

import ml_dtypes
import numpy as np

import concourse.bass as bass
import concourse.mybir as mybir
from concourse import bacc
from concourse.bass_utils import run_bass_kernel_spmd
from concourse.tile import TileContext

F32 = mybir.dt.float32
BF16 = mybir.dt.bfloat16
AFT = mybir.ActivationFunctionType
ALU = mybir.AluOpType

B, D, H = 8192, 1024, 1024
M_BATCH, M_UNIT = 2, 4          # batch split x unit split = 8 cores
BS = B // M_BATCH               # 4096 batch rows per core
U = H // M_UNIT                 # 256 hidden units per core
K = D + H                       # 2048 contraction (e_t | h_prev)
KT = K // 128                   # 16 k-chunks
G = 4 * U                       # 1024 gate rows per core (f|i|o|c x U)
BBLK = 512                      # moving free-dim per matmul
NBB = BS // BBLK                # 8 batch blocks
NJ = U // 128                   # 2 unit sub-blocks of 128 partitions

GATE_FUNCS = [AFT.Sigmoid, AFT.Sigmoid, AFT.Sigmoid, AFT.Tanh]  # f, i, o, c

N_WARMUP = 11


def _build_nc():
    nc = bacc.Bacc()

    xT = nc.dram_tensor("xT", [K, BS], BF16, kind="ExternalInput")
    wT = nc.dram_tensor("wT", [K, G], BF16, kind="ExternalInput")
    bias = nc.dram_tensor("bias", [128, 4 * NJ], F32, kind="ExternalInput")
    cT = nc.dram_tensor("cT", [U, BS], F32, kind="ExternalInput")
    # Outputs in bf16: adds ~2^-9 rounding (total err stays ~6e-3 vs the
    # 2e-2 gate) and halves output DMA bytes -- the final h_t transfer
    # is on the critical path after the last matmul.
    hT_out = nc.dram_tensor("hT_out", [U, BS], BF16, kind="ExternalOutput")
    cT_out = nc.dram_tensor("cT_out", [U, BS], BF16, kind="ExternalOutput")

    xT_r = xT.ap().rearrange("(k p) b -> p k b", p=128)      # [128, KT, BS]
    wT_r = wT.ap().rearrange("(k p) g -> p k g", p=128)      # [128, KT, G]
    cT_r = cT.ap().rearrange("(j p) b -> p j b", p=128)      # [128, NJ, BS]
    hT_r = hT_out.ap().rearrange("(j p) b -> p j b", p=128)
    cTo_r = cT_out.ap().rearrange("(j p) b -> p j b", p=128)

    with TileContext(nc) as tc:
        with (
            tc.tile_pool(name="wpool", bufs=1) as wpool,
            tc.tile_pool(name="xpool", bufs=2) as xpool,
            tc.tile_pool(name="cpool", bufs=2) as cpool,
            tc.tile_pool(name="gpool", bufs=2) as gpool,
            tc.tile_pool(name="opool", bufs=2) as opool,
            tc.tile_pool(name="psum", bufs=2, space="PSUM") as pp,
        ):
            # Bias arrives pre-tiled [128, 4*NJ] from the host (one 32B
            # row per partition, vs a 4096x4B gather).
            bias_sb = wpool.tile([128, 4 * NJ], F32)

            # PE warm-up on a zeroed scratch tile while the first W/x
            # chunks are in flight: trips the HAM clock gate's SHORT
            # window (~3.4us of sustained PE activity) early, so real
            # matmuls run at 2.4GHz from the first chunk.  The memset
            # runs on gpsimd, which exits the Tile prologue earliest.
            warm = wpool.tile([128, BBLK], BF16, name="warm")
            nc.gpsimd.memset(warm[:], 0.0)
            warm_ps = pp.tile([128, BBLK], F32, tag="ps0", name="warm_ps")
            for _ in range(N_WARMUP):
                nc.tensor.matmul(
                    warm_ps[:], warm[:, 0:128], warm[:], start=True, stop=True
                )

            # Per-k W tiles, DMA-interleaved with the first batch block's
            # x tiles so the PE can start on chunk 0 immediately and
            # chase the arrival stream.  Everything on the sync HWDGE
            # ring: its ~650ns/DMA issue rate paces arrivals in k-order;
            # the scalar ring is both slow (~55GB/s measured) and shares
            # the HWDGE sem-lane pool, and the scalar engine must stay
            # free for ACTIVATEs.  w0 lands as two halves, gate-hi half
            # first, so chunk 0's (c,o)-gate matmuls can start before
            # the (f,i) half arrives.
            w_sb = []
            x0_sb = []
            for k in range(KT):
                wt = wpool.tile([128, G], BF16, tag=f"w{k}", name=f"w{k}")
                if k == 0:
                    xp = xpool.tile(
                        [128, 2, BBLK], BF16, tag="xp0", name="xp0", bufs=3
                    )
                    nc.sync.dma_start(out=xp[:, 0, :], in_=xT_r[:, 0, 0:BBLK])
                    nc.sync.dma_start(out=wt[:, G // 2:], in_=wT_r[:, 0, G // 2:])
                    nc.sync.dma_start(out=wt[:, 0:G // 2], in_=wT_r[:, 0, 0:G // 2])
                else:
                    nc.sync.dma_start(out=wt[:], in_=wT_r[:, k, :])
                    if k % 2 == 0:
                        xp = xpool.tile(
                            [128, 2, BBLK], BF16, tag=f"xp{k // 2}", name=f"xp{k // 2}", bufs=3
                        )
                    nc.sync.dma_start(out=xp[:, k % 2, :], in_=xT_r[:, k, 0:BBLK])
                w_sb.append(wt)
                x0_sb.append(xp[:, k % 2, :])

            # bias + the first two cprev blocks ride the sync ring
            # BEHIND the W/x stream (engine FIFO defers them past the
            # chase window).  On gpsimd they would issue immediately and
            # the SDMA engines' packet round-robin between the gpsimd
            # and sync rings would steal ~half the early HBM bandwidth
            # exactly while the PE chases the k-stream (measured ~4us of
            # starvation).  cprev2+ stay on gpsimd, naturally gated
            # behind the previous block's output DMAs.
            nc.sync.dma_start(out=bias_sb[:], in_=bias.ap())

            def load_cprev(bb, ring=None):
                t = cpool.tile([128, NJ, BBLK], F32, tag="cprev", name="cprev")
                (ring or nc.gpsimd).dma_start(
                    out=t[:], in_=cT_r[:, :, bb * BBLK:(bb + 1) * BBLK]
                )
                return t

            def elementwise(ps, cprev_sb, bb, j, n_split=1, out_rings=None,
                            width=BBLK, coff=0):
                """Gate nonlinearities + cell update for one quadruple.

                ps[g] columns [0, width) correspond to batch-block
                columns [coff, coff+width).  out_rings: (ct, ht) ring
                per split chunk; default both gpsimd (SWDGE), fully
                overlapped in steady state.
                """
                w = width // n_split
                for s in range(n_split):
                    ct_ring, ht_ring = (
                        out_rings[s] if out_rings else (nc.gpsimd, nc.gpsimd)
                    )
                    psl = slice(s * w, (s + 1) * w)
                    csl = slice(coff + s * w, coff + (s + 1) * w)

                    def gate(g):
                        at = gpool.tile([128, w], F32, tag=f"act{g}", name=f"act{g}")
                        nc.scalar.activation(
                            at[:], ps[g][:, psl], GATE_FUNCS[g],
                            bias=bias_sb[:, 2 * g + j: 2 * g + j + 1],
                        )
                        return at

                    # ACT stream order mirrors the (c,f,i,o) matmul order so
                    # the c_t chain completes before the o-gate's matmuls do.
                    mc = gate(3)
                    gf = gate(0)
                    gi = gate(1)
                    t1 = gpool.tile([128, w], F32, tag="t1", name="t1")
                    nc.vector.tensor_tensor(
                        t1[:], gf[:], cprev_sb[:, j, csl], ALU.mult
                    )
                    t2 = gpool.tile([128, w], F32, tag="t2", name="t2")
                    nc.vector.tensor_tensor(t2[:], gi[:], mc[:], ALU.mult)
                    ct = opool.tile([128, w], BF16, tag="ct", name="ct")
                    nc.vector.tensor_tensor(ct[:], t1[:], t2[:], ALU.add)
                    th = gpool.tile([128, w], F32, tag="th", name="th")
                    nc.scalar.activation(th[:], ct[:], AFT.Tanh)
                    go = gate(2)
                    ht = opool.tile([128, w], BF16, tag="ht", name="ht")
                    nc.vector.tensor_tensor(ht[:], go[:], th[:], ALU.mult)

                    osl = slice(bb * BBLK + coff + s * w,
                                bb * BBLK + coff + (s + 1) * w)
                    ct_ring.dma_start(out=cTo_r[:, j, osl], in_=ct[:])
                    ht_ring.dma_start(out=hT_r[:, j, osl], in_=ht[:])

            # ---- batch block 0: k-outer over both j's, chasing the DMA
            # stream (8 MMs per arriving k-chunk vs ~1.4us/chunk
            # delivery) ----
            cprev0 = load_cprev(0, ring=nc.sync)
            ps0 = [
                [pp.tile([128, BBLK], F32, tag=f"ps{g}", name=f"ps{g}") for g in range(4)]
                for j in range(NJ)
            ]
            for k in range(KT):
                # chunk 0 runs j-inner with the upper-half gates (c,o)
                # first, matching the two-half w0 arrival order: the
                # first 4 matmuls need only w0-hi, buying ~1us for the
                # w0-lo half to land.  Elsewhere c-gate first (longest
                # elementwise chain).
                order = (
                    [(g, j) for g in (3, 2, 0, 1) for j in range(NJ)]
                    if k == 0 else
                    [(g, j) for j in range(NJ) for g in (3, 0, 1, 2)]
                )
                for g, j in order:
                    nc.tensor.matmul(
                        ps0[j][g][:],
                        w_sb[k][:, g * U + j * 128: g * U + (j + 1) * 128],
                        x0_sb[k][:],
                        start=(k == 0),
                        stop=(k == KT - 1),
                    )
            for j in range(NJ):
                elementwise(ps0[j], cprev0, 0, j)

            # ---- batch blocks 1..NBB-1: gate-outer, k-inner ----
            for bb in range(1, NBB):
                bsl = slice(bb * BBLK, (bb + 1) * BBLK)
                x_sb = []
                for kp in range(KT // 2):
                    xt = xpool.tile(
                        [128, 2, BBLK], BF16, tag=f"xp{kp}", name=f"xp{kp}", bufs=3
                    )
                    nc.sync.dma_start(out=xt[:], in_=xT_r[:, 2 * kp:2 * kp + 2, bsl])
                    x_sb.extend([xt[:, 0, :], xt[:, 1, :]])
                cprev_sb = load_cprev(bb, ring=nc.sync if bb == 1 else None)

                for j in range(NJ):
                    last = (bb == NBB - 1) and (j == NJ - 1)
                    if not last:
                        ps = [None] * 4
                        for g in (3, 0, 1, 2):  # c-gate first: longest chain
                            pst = pp.tile([128, BBLK], F32, tag=f"ps{g}", name=f"ps{g}")
                            col0 = g * U + j * 128
                            for k in range(KT):
                                nc.tensor.matmul(
                                    pst[:],
                                    w_sb[k][:, col0:col0 + 128],
                                    x_sb[k][:],
                                    start=(k == 0),
                                    stop=(k == KT - 1),
                                )
                            ps[g] = pst
                        elementwise(ps, cprev_sb, bb, j)
                        continue

                    # Final quadruple: two half-width (N=256) matmul
                    # groups in separate PSUM banks, so half 0's whole
                    # elementwise+output chain overlaps half 1's ~7us of
                    # matmuls and the post-last-matmul tail is just the
                    # o-gate ACT -> h_t mul -> one 64KB DMA.  ht rings
                    # split across sync/scalar so the two ~650ns DMA
                    # issues don't serialize.
                    hw = BBLK // 2
                    for h in range(2):
                        ps = [None] * 4
                        for g in (3, 0, 1, 2):
                            pst = pp.tile(
                                [128, BBLK], F32, tag=f"ps{g}", name=f"ps{g}h{h}"
                            )
                            col0 = g * U + j * 128
                            for k in range(KT):
                                nc.tensor.matmul(
                                    pst[:, 0:hw],
                                    w_sb[k][:, col0:col0 + 128],
                                    x_sb[k][:, h * hw:(h + 1) * hw],
                                    start=(k == 0),
                                    stop=(k == KT - 1),
                                )
                            ps[g] = pst
                        # h0 outputs on gpsimd/scalar (early, off the
                        # critical path); h1's ct+ht back-to-back on
                        # sync so the ht DMA hits a hot queue (an idle
                        # HWDGE ring adds ~0.6us first-byte latency)
                        # and both get fast HWDGE completion receipts.
                        elementwise(
                            ps, cprev_sb, bb, j,
                            out_rings=[(nc.gpsimd, nc.scalar) if h == 0
                                       else (nc.sync, nc.sync)],
                            width=hw, coff=h * hw,
                        )

    nc.finalize()
    return nc


def _shard_inputs(e_t, h_prev, c_prev, W_x, b_x, W_h, b_h, b_extra):
    e_t = np.asarray(e_t, dtype=np.float32)
    h_prev = np.asarray(h_prev, dtype=np.float32)
    c_prev = np.ascontiguousarray(np.asarray(c_prev, dtype=np.float32))
    W_x = np.asarray(W_x, dtype=np.float32)
    W_h = np.asarray(W_h, dtype=np.float32)
    bias_full = (
        np.asarray(b_x, dtype=np.float32)
        + np.asarray(b_h, dtype=np.float32)
        + np.asarray(b_extra, dtype=np.float32)
    )

    # X^T = [e_t | h_prev]^T : [K, B] in bf16
    XT = np.empty((K, B), dtype=ml_dtypes.bfloat16)
    XT[:D] = e_t.astype(ml_dtypes.bfloat16).T
    XT[D:] = h_prev.astype(ml_dtypes.bfloat16).T
    W = np.concatenate([W_x, W_h], axis=1).astype(ml_dtypes.bfloat16)  # [4H, K]

    in_maps = []
    for core in range(M_BATCH * M_UNIT):
        m, q = divmod(core, M_UNIT)
        rows = np.concatenate(
            [np.arange(g0 + q * U, g0 + (q + 1) * U) for g0 in (0, H, 2 * H, 3 * H)]
        )
        bias_tiled = np.ascontiguousarray(
            bias_full[rows].reshape(4, NJ, 128).transpose(2, 0, 1).reshape(128, 4 * NJ)
        )
        in_maps.append({
            "xT": np.ascontiguousarray(XT[:, m * BS:(m + 1) * BS]),
            "wT": np.ascontiguousarray(W[rows].T),
            "bias": bias_tiled,
            "cT": np.ascontiguousarray(c_prev[m * BS:(m + 1) * BS, q * U:(q + 1) * U].T),
        })
    return in_maps


def _assemble_outputs(results):
    h_t = np.empty((B, H), dtype=np.float32)
    c_t = np.empty((B, H), dtype=np.float32)
    for core, res in enumerate(results):
        m, q = divmod(core, M_UNIT)
        h_t[m * BS:(m + 1) * BS, q * U:(q + 1) * U] = np.asarray(
            res["hT_out"], dtype=np.float32).T
        c_t[m * BS:(m + 1) * BS, q * U:(q + 1) * U] = np.asarray(
            res["cT_out"], dtype=np.float32).T
    return h_t, c_t


def kernel(e_t, h_prev, c_prev, W_x, b_x, W_h, b_h, b_extra, _runner=None):
    in_maps = _shard_inputs(e_t, h_prev, c_prev, W_x, b_x, W_h, b_h, b_extra)
    nc = _build_nc()
    if _runner is None:
        res = run_bass_kernel_spmd(nc, in_maps, core_ids=list(range(8)))
        results = res.results
    else:
        results = _runner(nc, in_maps)
    return _assemble_outputs(results)
